# revision 13
# baseline (speedup 1.0000x reference)
"""GCN classifier on 8 TRN2 NeuronCores — v2.

Row-shards the 16384-node graph (2048 rows/core). The host supplies each
core its row-block of adj already TRANSPOSED (aT = adj[rows].T, [N, R]
fp32, j-major) so the device never transposes A: pass 1 is a pure stream
fp32 -> fp8 cast (DVE/ACT split) + column-degree ones-matmuls, with the
first NDR j-tiles written back to DRAM as fp8 and the last RES tiles kept
resident in SBUF. The unscaled XW1 is AllGathered in fp8 right after the
encoder so the collective fully overlaps pass-1 streaming; dis-scaling of
the gathered features happens consumer-side using a tiny (8 KiB/rank) dis
AllGather. Both aggregation passes run fp8 x fp8 DoubleRow matmuls
against the fp8 A tiles (resident or re-streamed), j-interleaved so DMA
and PE stay balanced. S2 is gathered pre-scaled in fp8.

Self-contained: hardcodes shapes for nn_GCNClassifer_6786048328674.
"""

import sys

sys.path.insert(0, "/opt/trn_rl_repo")

from contextlib import ExitStack

import numpy as np

import concourse.bass as bass
from concourse import bacc
import concourse.mybir as mybir
from concourse.tile import TileContext, add_dep_helper
from concourse.bass_utils import run_bass_kernel_spmd


F32 = mybir.dt.float32
BF16 = mybir.dt.bfloat16
FP8 = mybir.dt.float8e4
AF = mybir.ActivationFunctionType
ALU = mybir.AluOpType
DR = mybir.MatmulPerfMode.DoubleRow

BN_EPS = 1e-5
N_CORES = 8
P = 128
RES = 40                 # j-tiles of fp8 A kept resident in SBUF
GATHER_DT = FP8          # dtype for the two feature AllGathers
USE_DOUBLE_ROW = True
FSC = 64.0               # fp8 pre-scale so dis*feat avoids subnormals


def _interleave(a, b):
    """Proportionally merge two lists, preserving each one's order."""
    out, i, j = [], 0, 0
    while i < len(a) or j < len(b):
        if j >= len(b) or (i < len(a) and i * (len(b) + 1) <= j * (len(a) + 1)):
            out.append(a[i]); i += 1
        else:
            out.append(b[j]); j += 1
    return out


def build_nc(N=16384, F=1024, D1=512, E=256, H=256, G=128, C=10,
             n_cores=N_CORES, res=RES):
    R = N // n_cores            # rows (nodes) per core
    IC = R // 512               # 512-wide i chunks
    JT = N // P                 # 128-row j tiles
    NDR = JT - res              # j-tiles cached in DRAM
    assert NDR % 8 == 0 and res % 8 == 0
    GDT = GATHER_DT

    nc = bacc.Bacc(num_devices=n_cores)

    # ---- I/O -------------------------------------------------------------
    aT_d = nc.declare_dram_parameter("aT", [N, R], F32, isOutput=False)
    xT_d = nc.declare_dram_parameter("xT", [F, R], F32, isOutput=False)
    w1_d = nc.declare_dram_parameter("w1", [F, D1], F32, isOutput=False)
    k1_d = nc.declare_dram_parameter("k1", [D1], F32, isOutput=False)
    c1_d = nc.declare_dram_parameter("c1", [D1], F32, isOutput=False)
    w2_d = nc.declare_dram_parameter("w2", [D1, E], F32, isOutput=False)
    k2_d = nc.declare_dram_parameter("k2", [E], F32, isOutput=False)
    c2_d = nc.declare_dram_parameter("c2", [E], F32, isOutput=False)
    g1w_d = nc.declare_dram_parameter("g1w", [E, H], F32, isOutput=False)
    g1b_d = nc.declare_dram_parameter("g1b", [H], F32, isOutput=False)
    g2w_d = nc.declare_dram_parameter("g2w", [H, G], F32, isOutput=False)
    g2b_d = nc.declare_dram_parameter("g2b", [G], F32, isOutput=False)
    cw_d = nc.declare_dram_parameter("cw", [G, C], F32, isOutput=False)
    cb_d = nc.declare_dram_parameter("cb", [C], F32, isOutput=False)
    idn_d = nc.declare_dram_parameter("idn", [P, P], F32, isOutput=False)
    out_d = nc.declare_dram_parameter("out", [C, R], F32, isOutput=True)

    # ---- collective DRAM tensors ----------------------------------------
    u1_loc = nc.dram_tensor("u1_loc", [R, E], GDT)
    U1g = nc.dram_tensor("U1g", [N, E], GDT, addr_space="Shared")
    dis_loc = nc.dram_tensor("dis_loc", [1, R], F32)
    disG = nc.dram_tensor("disG", [n_cores, R], F32, addr_space="Shared")
    s2_loc = nc.dram_tensor("s2_loc", [R, G], GDT)
    S2g = nc.dram_tensor("S2g", [N, G], GDT, addr_space="Shared")
    groups = [list(range(n_cores))]

    with TileContext(nc) as tc, ExitStack() as ctx:
        wpool = ctx.enter_context(tc.tile_pool(name="wpool", bufs=1))
        dram = ctx.enter_context(tc.tile_pool(name="dram", bufs=1, space="DRAM"))
        psum = ctx.enter_context(tc.tile_pool(name="psum", bufs=1, space="PSUM"))
        _psn = [0]

        def ps_tile(shape, dtype, banks=range(8), name=None):
            tag = "b%d" % (list(banks)[_psn[0] % len(list(banks))])
            _psn[0] += 1
            return psum.tile(shape, dtype, tag=tag, name=name or f"ps{_psn[0]}")

        # ---- constants / weights in SBUF --------------------------------
        idb = wpool.tile([P, P], BF16)
        nc.gpsimd.dma_start(idb, idn_d[:, :])
        idf = wpool.tile([P, P], F32)
        nc.sync.dma_start(idf, idn_d[:, :])
        ones_b = wpool.tile([P, 1], BF16)
        nc.vector.memset(ones_b, 1.0)

        w1_sb = wpool.tile([P, F // P, D1], BF16)
        nc.gpsimd.dma_start(w1_sb, w1_d.ap().rearrange("(ko p) m -> p ko m", p=P))
        w2_sb = wpool.tile([P, D1 // P, E], BF16)
        nc.gpsimd.dma_start(w2_sb, w2_d.ap().rearrange("(ko p) m -> p ko m", p=P))
        g1w_sb = wpool.tile([P, E // P, H], BF16)
        nc.gpsimd.dma_start(g1w_sb, g1w_d.ap().rearrange("(ko p) m -> p ko m", p=P))
        g2w_sb = wpool.tile([P, H // P, G], BF16)
        nc.gpsimd.dma_start(g2w_sb, g2w_d.ap().rearrange("(ko p) m -> p ko m", p=P))
        cw_sb = wpool.tile([G, C], BF16)
        nc.gpsimd.dma_start(cw_sb, cw_d[:, :])

        def load_vec(d, n, nm):
            t = wpool.tile([P, n // P], F32, tag=nm, name=nm)
            nc.sync.dma_start(t, d.ap().rearrange("(o p) -> p o", p=P))
            return t

        k1_sb = load_vec(k1_d, D1, "k1v")
        c1_sb = load_vec(c1_d, D1, "c1v")
        k2_sb = load_vec(k2_d, E, "k2v")
        c2_sb = load_vec(c2_d, E, "c2v")
        g1b_sb = load_vec(g1b_d, H, "g1bv")
        g2b_sb = load_vec(g2b_d, G, "g2bv")
        cb_sb = wpool.tile([C, 1], F32)
        nc.sync.dma_start(cb_sb, cb_d.ap().rearrange("(c o) -> c o", o=1))

        # ---- persistent SBUF tensors -------------------------------------
        ares = wpool.tile([P, res, R], FP8, tag="ares")    # resident A^T tiles
        u1_sb = wpool.tile([P, H // P, R], BF16)           # xw1, feature-major
        h3_sb = wpool.tile([P, H // P, R], BF16)
        h4_sb = wpool.tile([P, R], BF16)
        dis_bc = wpool.tile([P, R], F32)
        disg_sb = wpool.tile([P, JT], F32)                 # dis[jt*128+p] at [p, jt]
        out_sb = wpool.tile([C, R], F32)
        dmy = wpool.tile([1, R], F32)

        a8d = dram.tile([NDR * P, R], FP8)
        a8r = a8d.rearrange("(t p) i -> p t i", p=P)
        dis_dram = dram.tile([1, R], F32)

        # =========== encoder: h1, h2, u1 = xw1 (feature-major) ===========
        xT_r = xT_d.ap().rearrange("(ko p) i -> p ko i", p=P)
        with tc.tile_pool(name="enc_io", bufs=2) as enc_io:
            for s in range(IC):
                isl = bass.ts(s, 512)
                xs = enc_io.tile([P, F // P, 512], BF16, tag="xstrip")
                nc.gpsimd.dma_start(xs, xT_r[:, :, isl])
                h1s = enc_io.tile([P, D1 // P, 512], BF16, tag="h1s")
                for m in range(D1 // P):
                    ps = ps_tile([P, 512], F32, banks=range(4))
                    for k in range(F // P):
                        nc.tensor.matmul(ps, w1_sb[:, k, bass.ts(m, P)], xs[:, k],
                                         start=(k == 0), stop=(k == F // P - 1))
                    nc.scalar.activation(h1s[:, m], ps, AF.Relu,
                                         bias=c1_sb[:, m:m + 1], scale=k1_sb[:, m:m + 1])
                h2s = enc_io.tile([P, E // P, 512], BF16, tag="h2s")
                for m in range(E // P):
                    ps = ps_tile([P, 512], F32, banks=range(4))
                    for k in range(D1 // P):
                        nc.tensor.matmul(ps, w2_sb[:, k, bass.ts(m, P)], h1s[:, k],
                                         start=(k == 0), stop=(k == D1 // P - 1))
                    nc.scalar.activation(h2s[:, m], ps, AF.Relu,
                                         bias=c2_sb[:, m:m + 1], scale=k2_sb[:, m:m + 1])
                for m in range(H // P):
                    ps = ps_tile([P, 512], F32, banks=range(4))
                    for k in range(E // P):
                        nc.tensor.matmul(ps, g1w_sb[:, k, bass.ts(m, P)], h2s[:, k],
                                         start=(k == 0), stop=(k == E // P - 1))
                    nc.vector.tensor_copy(u1_sb[:, m, isl], ps)

            # u1 -> natural layout fp8 -> AllGather (unscaled!)
            for ib in range(R // 512):
                snat = enc_io.tile([P, 4, E], GDT, tag="snat")
                for b in range(4):
                    it = ib * 4 + b
                    pst = ps_tile([P, E], BF16, banks=range(4))
                    for m in range(H // P):
                        nc.tensor.transpose(pst[:, bass.ts(m, P)],
                                            u1_sb[:, m, bass.ts(it, P)], idb)
                    nc.scalar.activation(snat[:, b], pst, AF.Copy)
                nc.gpsimd.dma_start(
                    u1_loc[bass.ts(ib, 512), :].rearrange("(t p) e -> p t e", p=P),
                    snat)
        cc1 = nc.gpsimd.collective_compute(
            "AllGather", ALU.bypass, replica_groups=groups,
            ins=[u1_loc[:, :].opt()], outs=[U1g[:, :].opt()])

        # =========== pass 1: stream A^T fp32 -> fp8 + degrees =============
        aT_r = aT_d.ap().rearrange("(t p) i -> p t i", p=P)
        dps = [psum.tile([1, 512], F32, tag=f"b{4 + i}", name=f"degps{i}")
               for i in range(IC)]
        with tc.tile_pool(name="p1_io", bufs=3) as p1_io:
            for jt in range(JT):
                at = p1_io.tile([P, R], F32, tag="a_in")
                eng_d = nc.sync if jt % 2 == 0 else nc.scalar
                eng_w = nc.scalar if jt % 2 == 0 else nc.sync
                eng_d.dma_start(at, aT_r[:, jt, :])
                if jt < NDR:
                    af8 = p1_io.tile([P, R], FP8, tag="a_f8")
                else:
                    af8 = ares[:, jt - NDR, :]
                if jt % 2 == 0:
                    nc.vector.tensor_copy(af8, at)
                else:
                    nc.scalar.activation(af8, at, AF.Copy)
                for i in range(IC):
                    nc.tensor.matmul(dps[i], ones_b, af8[:, bass.ts(i, 512)],
                                     start=(jt == 0), stop=(jt == JT - 1))
                if jt < NDR:
                    # HWDGE, not gpsimd: SWDGE lanes FIFO behind the in-flight
                    # AllGather and would stall the whole stream
                    eng_w.dma_start(a8r[:, jt, :], af8)

        # dis = deg^-0.5 (own rows; no collective needed for own dis)
        for i in range(IC):
            nc.vector.tensor_copy(dmy[:, bass.ts(i, 512)], dps[i])
        nc.vector.reciprocal(dmy, dmy)
        nc.scalar.activation(dmy, dmy, AF.Sqrt)
        nc.sync.dma_start(dis_dram, dmy)
        nc.sync.dma_start(dis_bc, dis_dram[0:1, :].to_broadcast([P, R]))
        nc.sync.dma_start(dis_loc[:, :], dmy)
        ccd = nc.gpsimd.collective_compute(
            "AllGather", ALU.bypass, replica_groups=groups,
            ins=[dis_loc[:, :].opt()], outs=[disG[:, :].opt()])

        # global dis, j-tile-major: disg_sb[p, t] = dis[t*128 + p]
        disg_nat = wpool.tile([P, P], F32)
        dgl = nc.sync.dma_start(disg_nat,
                                disG.ap().rearrange("c (t p) -> (c t) p", p=P))
        add_dep_helper(dgl.ins, ccd.ins, reason="disG read after AG")
        pst_d = ps_tile([P, P], F32, banks=range(4))
        nc.tensor.transpose(pst_d, disg_nat, idf)
        # FSC folded in: lhsT tiles become FSC*dis_j*u1; epilogues divide out
        nc.vector.tensor_scalar(disg_sb, pst_d, FSC, None, ALU.mult)

        # =========== pass 2: y1 = (A @ (dis*U1))^T, h3 = relu(dis*y1+b) ===
        NB1 = H // P
        U1_r = U1g.ap().rearrange("(t p) e -> p t e", p=P)
        S2_r = S2g.ap().rearrange("(t p) g -> p t g", p=P)
        ps_y = [psum.tile([P, 512], F32, tag=f"b{m * IC + i}",
                          name=f"ps_y_{m}_{i}")
                for m in range(NB1) for i in range(IC)]
        # group = 8 consecutive j-tiles; interleave DRAM and resident groups
        order = _interleave([('d', g) for g in range(NDR // 8)],
                            [('r', g) for g in range(res // 8)])
        pos = {}
        for gi, key in enumerate(order):
            pos[key] = gi

        def agg_pass(sg_r, sg_dt, sdim, nb, ps_list, cc_feat, scale_feat):
            """One aggregation pass over all j-tiles (grouped, interleaved)."""
            with tc.tile_pool(name="agg_io", bufs=3) as agg_io, \
                 tc.tile_pool(name="agg_s", bufs=2) as agg_s:
                for gi, (kind, g) in enumerate(order):
                    jt0 = g * 8 if kind == 'd' else NDR + g * 8
                    st = agg_s.tile([P, 8, sdim], sg_dt, tag="sg_in")
                    dse = nc.scalar.dma_start(st, sg_r[:, jt0:jt0 + 8, :])
                    add_dep_helper(dse.ins, cc_feat.ins, reason="feat after AG")
                    if kind == 'd':
                        a_lo = agg_io.tile([P, 4, R], FP8, tag="a8_lo")
                        nc.sync.dma_start(a_lo, a8r[:, g * 8:g * 8 + 4, :])
                        a_hi = agg_io.tile([P, 4, R], FP8, tag="a8_hi")
                        nc.sync.dma_start(a_hi, a8r[:, g * 8 + 4:g * 8 + 8, :])
                    for q in range(4):          # pairs within group
                        jp = jt0 + 2 * q
                        if kind == 'd':
                            rhs = (a_lo if q < 2 else a_hi)[:, (2 * q) % 4:(2 * q) % 4 + 2, :]
                        else:
                            rhs = ares[:, jp - NDR:jp - NDR + 2, :]
                        if scale_feat:
                            sc = agg_s.tile([P, 2, sdim], FP8, tag="sg_sc")
                            for h in range(2):
                                nc.scalar.activation(
                                    sc[:, h], st[:, 2 * q + h], AF.Copy,
                                    scale=disg_sb[:, jp + h:jp + h + 1])
                        else:
                            sc = st[:, 2 * q:2 * q + 2, :]
                        first = (gi == 0 and q == 0)
                        last = (gi == len(order) - 1 and q == 3)
                        for m in range(nb):
                            for i in range(IC):
                                if USE_DOUBLE_ROW:
                                    nc.tensor.matmul(
                                        ps_list[m * IC + i],
                                        sc[:, :, bass.ts(m, P)],
                                        rhs[:, :, bass.ts(i, 512)],
                                        start=first, stop=last, perf_mode=DR)
                                else:
                                    for h in range(2):
                                        nc.tensor.matmul(
                                            ps_list[m * IC + i],
                                            sc[:, h, bass.ts(m, P)],
                                            rhs[:, h, bass.ts(i, 512)],
                                            start=(first and h == 0),
                                            stop=(last and h == 1))

        agg_pass(U1_r, GDT, E, NB1, ps_y, cc1, True)

        with tc.tile_pool(name="ep2", bufs=2) as ep2:
            for m in range(NB1):
                for i in range(IC):
                    isl = bass.ts(i, 512)
                    tt = ep2.tile([P, 512], F32, tag="ep")
                    nc.vector.tensor_tensor(tt, ps_y[m * IC + i], dis_bc[:, isl],
                                            ALU.mult)
                    nc.scalar.activation(h3_sb[:, m, isl], tt, AF.Relu,
                                         bias=g1b_sb[:, m:m + 1], scale=1.0 / FSC)

            # xw2, s2 = dis*xw2 -> natural fp8 -> AllGather
            s2T_sb = wpool.tile([P, R], BF16)
            for i in range(IC):
                isl = bass.ts(i, 512)
                ps = ps_tile([P, 512], F32, banks=range(4))
                for k in range(H // P):
                    nc.tensor.matmul(ps, g2w_sb[:, k, :], h3_sb[:, k, isl],
                                     start=(k == 0), stop=(k == H // P - 1))
                tt2 = ep2.tile([P, 512], F32, tag="ep")
                nc.vector.tensor_tensor(tt2, ps, dis_bc[:, isl], ALU.mult)
                nc.vector.tensor_scalar(s2T_sb[:, isl], tt2, FSC, None, ALU.mult)
            for ib in range(R // 512):
                snat2 = ep2.tile([P, 4, G], GDT, tag="snat2")
                for b in range(4):
                    it = ib * 4 + b
                    pst = ps_tile([P, G], BF16, banks=range(4))
                    nc.tensor.transpose(pst, s2T_sb[:, bass.ts(it, P)], idb)
                    nc.scalar.activation(snat2[:, b], pst, AF.Copy)
                nc.gpsimd.dma_start(
                    s2_loc[bass.ts(ib, 512), :].rearrange("(t p) g -> p t g", p=P),
                    snat2)
        cc2 = nc.gpsimd.collective_compute(
            "AllGather", ALU.bypass, replica_groups=groups,
            ins=[s2_loc[:, :].opt()], outs=[S2g[:, :].opt()])

        # =========== pass 3: y2 = (A @ s2g)^T, h4 = relu(dis*y2+b) ========
        ps_z = [psum.tile([P, 512], F32, tag=f"b{4 + i}", name=f"ps_z_{i}")
                for i in range(IC)]
        agg_pass(S2_r, GDT, G, 1, ps_z, cc2, False)

        with tc.tile_pool(name="ep3", bufs=2) as ep3:
            for i in range(IC):
                isl = bass.ts(i, 512)
                tt = ep3.tile([P, 512], F32, tag="ep")
                nc.vector.tensor_tensor(tt, ps_z[i], dis_bc[:, isl], ALU.mult)
                nc.scalar.activation(h4_sb[:, isl], tt, AF.Relu,
                                     bias=g2b_sb[:, 0:1], scale=1.0 / FSC)

            # classifier: out = clip(sigmoid(h4 @ cw + cb))
            for i in range(IC):
                isl = bass.ts(i, 512)
                ps = ps_tile([C, 512], F32, banks=range(4))
                nc.tensor.matmul(ps, cw_sb, h4_sb[:, isl], start=True, stop=True)
                nc.scalar.activation(out_sb[:, isl], ps, AF.Sigmoid, bias=cb_sb)
        nc.vector.tensor_scalar(out_sb, out_sb, 1.0 - 1e-10, 1e-10,
                                ALU.min, ALU.max)
        nc.sync.dma_start(out_d[:, :], out_sb)

    nc.finalize()
    return nc


def make_in_maps(inputs, N, n_cores=N_CORES):
    f = {k: np.ascontiguousarray(np.asarray(v, dtype=np.float32))
         for k, v in inputs.items()}
    k1 = f["bn1_g"] / np.sqrt(f["bn1_v"] + BN_EPS)
    c1 = (f["enc_b1"] - f["bn1_m"]) * k1 + f["bn1_b"]
    k2 = f["bn2_g"] / np.sqrt(f["bn2_v"] + BN_EPS)
    c2 = (f["enc_b2"] - f["bn2_m"]) * k2 + f["bn2_b"]
    R = N // n_cores
    shared = dict(
        w1=f["enc_w1"], k1=k1, c1=c1,
        w2=f["enc_w2"], k2=k2, c2=c2,
        g1w=f["gcn1_w"], g1b=f["gcn1_b"],
        g2w=f["gcn2_w"], g2b=f["gcn2_b"],
        cw=f["cls_w"], cb=f["cls_b"],
        idn=np.eye(128, dtype=np.float32),
    )
    maps = []
    for c in range(n_cores):
        r0, r1 = c * R, (c + 1) * R
        m = dict(shared)
        m["aT"] = np.ascontiguousarray(f["adj"][r0:r1].T)
        m["xT"] = np.ascontiguousarray(f["feature"][r0:r1].T)
        maps.append(m)
    return maps


_NC_CACHE = {}


def run(inputs, trace=False, N=16384, n_cores=N_CORES):
    key = (N, n_cores)
    if key not in _NC_CACHE:
        _NC_CACHE[key] = build_nc(N=N, n_cores=n_cores)
    nc = _NC_CACHE[key]
    in_maps = make_in_maps(inputs, N, n_cores)
    res = run_bass_kernel_spmd(nc, in_maps, core_ids=list(range(n_cores)),
                               trace=trace)
    out = np.concatenate([r["out"].T for r in res.results], axis=0)
    return np.ascontiguousarray(out.astype(np.float32)), res


def kernel(**inputs) -> np.ndarray:
    out, _ = run(inputs, trace=False)
    return out
